# revision 41
# baseline (speedup 1.0000x reference)
"""Batched KNN (k=16) + mean feature gather on 8 Trainium2 NeuronCores.

Problem: for each of 16384 query points x (3-D), find the 16 nearest
neighbors among 16384 base points y restricted to the same batch id, and
output the mean of their 16-D features.

Strategy (one core per 2048-query shard; batch-sorted ids give per-batch
locality so each core only needs its own y span — no collectives):

1. Scores S[i,j] = 2*x_i.y_j - |y_j|^2 (row-constant -|x|^2 dropped; order
   preserved) via TensorE matmul in bf16 with 3-term split arithmetic
   (f32-accurate), plus a batch-mismatch penalty -65536*(xb-yb)^2 folded in
   as extra contraction slots (exactly cancels for same-batch pairs).
   Contraction is zero-padded to K=128: a partially-filled PE array keeps
   the HAM clock gate cold (measured 2x slower).
2. Per-row top-16 threshold on VectorE: max8 per 256-wide group, then
   merge the 8*G group candidates with max8/match_replace to get the 16th
   and 17th largest; threshold t = midpoint.
3. D = S^T - t via a second matmul (j on partitions) with -t as 3 extra
   bf16-split contraction slots (t transposed via a small DRAM roundtrip);
   selection weights from D: ScalarE Sign -> +/-1 (VectorE (D>0)*2 -> {0,2}
   on some chunks of the last quarter, where VectorE is otherwise idle).
4. Gather: gT[f, i] = sum_j feat[j, f] * w[j, i] on TensorE with feats as
   the stationary operand; out = (gT + colsum_of_sign_chunks)/32, then a
   PE transpose back to row-major and a contiguous store.
Queries are processed in 4 quarters so phase 2 (DVE-bound) of one quarter
overlaps phases 3/4 (PE/ACT-bound) of the previous one.
"""

import os

import numpy as np
import ml_dtypes

import concourse.bass as bass
import concourse.mybir as mybir
from concourse import bacc
from concourse.tile import TileContext
from concourse.bass_utils import run_bass_kernel_spmd

N_CORES = 8
FEAT = 16
PEN = 65536.0
SENTINEL = 16.0  # batch id for padded y rows (real ids are < 8)
NEG_BIG = -3.0e38
NH = 4           # query quarters (phase A/C software pipeline)

bf16 = ml_dtypes.bfloat16

# contraction slot layout
KS = 3 + 3 + 18  # penalty + y^2 splits + 6 product terms per coordinate
T0 = 32          # threshold rows start here (DMA-to-SBUF needs start % 32 == 0)
KD = T0 + 3      # + 3 threshold split slots (S^T - t matmul only)


def _act_chunk(jc, quarter):
    """Which engine evicts the selection weights for candidate chunk jc of
    this quarter. True -> ScalarE Sign (+/-1, counted in the colsum
    correction); False -> VectorE (D>0)*2 ({0,2}, no correction)."""
    return not (quarter == NH - 1 and jc % 2 == 1)


def _split3(v):
    """3-term bf16 split of a float64 array: v ~ h+m+l, residual ~2^-27 |v|."""
    h = v.astype(bf16)
    r = v - h.astype(np.float64)
    m = r.astype(bf16)
    l = (r - m.astype(np.float64)).astype(bf16)
    return h, m, l


def _build_sides(xc, xbc, yc, ybc):
    """Host prep of the contraction-slot tensors.

    Returns (X [KD, R], Y [KD, C]) bf16. X rows T0..KD-1 are zeros (filled
    on device with the -t splits); Y rows T0..KD-1 are ones.
    """
    R, C = xc.shape[0], yc.shape[0]
    xs, ys = [], []
    xb64 = xbc.astype(np.float64)
    yb64 = ybc.astype(np.float64)
    # batch penalty: accumulates first, exactly cancels when xb == yb
    xs += [-PEN * xb64 * xb64, 2 * PEN * xb64, np.full(R, -PEN)]
    ys += [np.ones(C), yb64, yb64 * yb64]
    # -|y|^2, 3-split
    c = -(yc.astype(np.float64) ** 2).sum(1)
    ch, cm, cl = (t.astype(np.float64) for t in _split3(c))
    xs += [np.ones(R)] * 3
    ys += [ch, cm, cl]
    # products 2*x_k*y_k, 6 split terms per coordinate
    for k in range(3):
        a = 2.0 * xc[:, k].astype(np.float64)
        b = yc[:, k].astype(np.float64)
        ah, am, al = (t.astype(np.float64) for t in _split3(a))
        bh, bm, bl = (t.astype(np.float64) for t in _split3(b))
        for xa, yb_ in [(ah, bh), (ah, bm), (am, bh), (ah, bl), (al, bh), (am, bm)]:
            xs.append(xa)
            ys.append(yb_)
    # zero padding up to T0, then device-filled threshold slots (y side = 1)
    while len(xs) < T0:
        xs.append(np.zeros(R))
        ys.append(np.zeros(C))
    xs += [np.zeros(R)] * 3
    ys += [np.ones(C)] * 3
    X = np.stack([v.astype(bf16) for v in xs])
    Y = np.stack([v.astype(bf16) for v in ys])
    return X, Y


def _build_nc(R, C):
    """Build the Bass graph for one core (SPMD: all cores run this)."""
    rb = R // 128    # query row blocks
    G = C // 128     # candidate chunks (gather/selection granularity)
    GW = 256         # max8 group width
    RH = R // NH     # rows per quarter
    hs = rb // NH    # row blocks per quarter
    f32 = mybir.dt.float32
    bft = mybir.dt.bfloat16

    nc = bacc.Bacc(name="knn16")
    xk = nc.dram_tensor("xk", [KD, R], bft, kind="ExternalInput")
    yk = nc.dram_tensor("yk", [KD, C], bft, kind="ExternalInput")
    fe = nc.dram_tensor("fe", [C, FEAT], bft, kind="ExternalInput")
    cs = nc.dram_tensor("cs", [FEAT, NH], f32, kind="ExternalInput")
    td = nc.dram_tensor("td", [3 * R], bft, kind="Internal")
    out = nc.dram_tensor("out", [R, FEAT], f32, kind="ExternalOutput")

    with TileContext(nc) as tc:
        with (
            tc.tile_pool(name="const", bufs=1) as const,
            tc.tile_pool(name="spool", bufs=2, space="PSUM") as spool,
            tc.tile_pool(name="dpool", bufs=3, space="PSUM") as dpool,
            tc.tile_pool(name="gpool", bufs=2, space="PSUM") as gpool,
            tc.tile_pool(name="trpool", bufs=1, space="PSUM") as trpool,
            tc.tile_pool(name="work", bufs=2) as work,
            tc.tile_pool(name="wpool", bufs=3) as wpool,
        ):
            # full 128 contraction rows (rows >= KD stay zero, see K-pad note)
            # xk is split per quarter so a quarter's t-row readback (write)
            # never serializes against the next quarter's score matmuls.
            xk_q = [
                const.tile([128, R // NH], bft, name=f"xkq{q}", tag=f"xkq{q}")
                for q in range(NH)
            ]
            yk_sb = const.tile([128, C], bft)
            fe_sb = const.tile([128, G * FEAT], bft)
            cs_sb = const.tile([FEAT, NH], f32)
            id_sb = const.tile([16, 16], f32)
            # tr: one PSUM bank collecting the transposed [128, 16] output
            # chunks; gT: per-quarter [16, 512] out^T accumulator bank. Both
            # are cleared once by a zero-weight matmul so the real matmuls
            # accumulate (start=False) and never bank-clear each other.
            tr = trpool.tile([128, 512], f32, tag="tr")

            for q in range(NH):
                nc.vector.memset(xk_q[q][:, :], 0.0)
            nc.vector.memset(yk_sb[:, :], 0.0)
            for q in range(C // 512):
                nc.sync.dma_start(
                    out=yk_sb[0:KD, q * 512:(q + 1) * 512],
                    in_=yk[:, q * 512:(q + 1) * 512],
                )
            for q in range(NH):
                nc.sync.dma_start(
                    out=xk_q[q][0:T0, :],
                    in_=xk[0:T0, q * (R // NH):(q + 1) * (R // NH)],
                )
            nc.sync.dma_start(
                out=fe_sb[:, :].rearrange("p (g f) -> p g f", g=G),
                in_=fe[:, :].rearrange("(g p) f -> p g f", p=128),
            )
            nc.sync.dma_start(out=cs_sb[:, :], in_=cs[:, :])
            from concourse.masks import make_identity

            make_identity(nc, id_sb)
            zz_sb = const.tile([1, 512], bft)
            nc.vector.memset(zz_sb, 0.0)

            def zero_bank(zb):
                nc.tensor.matmul(
                    zb,
                    lhsT=zz_sb[0:1, 0:128],
                    rhs=zz_sb[0:1, 0:512],
                    start=True,
                    stop=False,
                    skip_group_check=True,
                )

            zero_bank(tr)

            for qr in range(NH):
                b_lo, b_hi = qr * hs, (qr + 1) * hs
                # phase A "blocks" are strided column sets of this quarter:
                # block b covers xk columns qr*RH + p*hs + b (p = 0..127),
                # which makes the t scatter's last dim contiguous.
                xk_str = xk_q[qr][0:128, :].rearrange("k (p b) -> k b p", b=hs)
                t_all = work.tile([128, hs], f32, tag="tall")
                # ---- Phase A: scores + per-row top-16/17 values ----
                for b in range(b_lo, b_hi):
                    cand = work.tile([128, (C // GW) * 8], f32, tag="cand")
                    for q in range(C // 512):
                        s_ps = spool.tile([128, 512], f32, tag="S")
                        nc.tensor.matmul(
                            s_ps,
                            lhsT=xk_str[:, b - b_lo, :],
                            rhs=yk_sb[0:128, q * 512:(q + 1) * 512],
                            start=True,
                            stop=True,
                        )
                        for g in range(512 // GW):
                            gi = q * (512 // GW) + g
                            nc.vector.max(
                                out=cand[:, gi * 8:(gi + 1) * 8],
                                in_=s_ps[:, g * GW:(g + 1) * GW],
                            )
                    # merge: 16th + 17th largest of the group winners
                    m1 = work.tile([128, 8], f32, tag="m1")
                    nc.vector.max(out=m1, in_=cand)
                    cand2 = work.tile([128, (C // GW) * 8], f32, tag="cand2")
                    nc.vector.match_replace(
                        out=cand2, in_to_replace=m1, in_values=cand,
                        imm_value=NEG_BIG,
                    )
                    m2 = work.tile([128, 8], f32, tag="m2")
                    nc.vector.max(out=m2, in_=cand2)
                    cand3 = work.tile([128, (C // GW) * 8], f32, tag="cand3")
                    nc.vector.match_replace(
                        out=cand3, in_to_replace=m2, in_values=cand2,
                        imm_value=NEG_BIG,
                    )
                    v17 = work.tile([128, 1], f32, tag="v17")
                    nc.vector.tensor_reduce(
                        out=v17, in_=cand3, axis=mybir.AxisListType.X,
                        op=mybir.AluOpType.max,
                    )
                    nc.vector.tensor_add(
                        out=t_all[:, b - b_lo:b - b_lo + 1], in0=m2[:, 7:8],
                        in1=v17,
                    )

                # batched threshold split: tneg = -(v16+v17)/2 as 3 bf16 terms
                hb = slice(0, hs)
                tsplit = work.tile([128, 3, hs], bft, tag="tsplit")
                th_ = tsplit[:, 0, :]
                tm_ = tsplit[:, 1, :]
                tl_ = tsplit[:, 2, :]
                r1 = work.tile([128, hs], f32, tag="r1")
                r2 = work.tile([128, hs], f32, tag="r2")
                rt = work.tile([128, hs], f32, tag="rt")
                nc.vector.tensor_scalar_mul(th_, t_all[:, hb], -0.5)
                nc.vector.tensor_copy(out=rt, in_=th_)
                nc.vector.scalar_tensor_tensor(
                    out=r1, in0=t_all[:, hb], scalar=-0.5, in1=rt,
                    op0=mybir.AluOpType.mult, op1=mybir.AluOpType.subtract,
                )
                nc.vector.tensor_copy(out=tm_, in_=r1)
                nc.vector.tensor_copy(out=rt, in_=tm_)
                nc.vector.tensor_sub(out=r2, in0=r1, in1=rt)
                nc.vector.tensor_copy(out=tl_, in_=r2)

                # ---- Phase B: transpose tneg via DRAM roundtrip ----
                # td flat layout: addr = s*R + qr*RH + p*hs + b
                with nc.allow_non_contiguous_dma("t transpose scatter"):
                    nc.sync.dma_start(
                        out=bass.AP(td, qr * RH, [[hs, 128], [R, 3], [1, hs]]),
                        in_=tsplit[:, :, :],
                    )
                nc.sync.dma_start(
                    out=xk_q[qr][T0:KD, :],
                    in_=bass.AP(td, qr * RH, [[R, 3], [1, RH]]),
                )

                # ---- Phase C: D = S^T - t, selection weights, gather ----
                gT = gpool.tile([128, 512], f32, tag="gT")
                zero_bank(gT)
                i0 = qr * RH
                for jc in range(G):
                    d_ps = dpool.tile([128, 512], f32, tag="D")
                    nc.tensor.matmul(
                        d_ps,
                        lhsT=yk_sb[0:128, jc * 128:(jc + 1) * 128],
                        rhs=xk_q[qr][0:128, :],
                        start=True,
                        stop=True,
                    )
                    w_sb = wpool.tile([128, RH], bft, tag="W")
                    if _act_chunk(jc, qr):
                        nc.scalar.activation(
                            out=w_sb,
                            in_=d_ps,
                            func=mybir.ActivationFunctionType.Sign,
                        )
                    else:
                        nc.vector.tensor_scalar(
                            out=w_sb,
                            in0=d_ps,
                            scalar1=0.0,
                            scalar2=2.0,
                            op0=mybir.AluOpType.is_gt,
                            op1=mybir.AluOpType.mult,
                        )
                    nc.tensor.matmul(
                        gT[0:16, :],
                        lhsT=fe_sb[:, jc * FEAT:(jc + 1) * FEAT],
                        rhs=w_sb,
                        start=False,
                        stop=(jc == G - 1),
                        skip_group_check=True,
                    )

                # ---- Phase D: epilogue (gT + colsum)/32, transpose, store ----
                outT = work.tile([16, RH], f32, tag="outT")
                nc.vector.tensor_scalar(
                    out=outT[:, :],
                    in0=gT[0:16, :],
                    scalar1=cs_sb[:, qr:qr + 1],
                    scalar2=1.0 / 32.0,
                    op0=mybir.AluOpType.add,
                    op1=mybir.AluOpType.mult,
                )
                for k in range(RH // 128):
                    slot = qr * (RH // 128) + k
                    nc.tensor.matmul(
                        tr[:, slot * FEAT:(slot + 1) * FEAT],
                        lhsT=outT[:, k * 128:(k + 1) * 128],
                        rhs=id_sb,
                        is_transpose=True,
                        start=False,
                        stop=False,
                        skip_group_check=True,
                    )
                ob = work.tile([128, RH // 8], f32, tag="ob")
                nc.vector.tensor_copy(
                    out=ob, in_=tr[:, qr * RH // 8:(qr + 1) * RH // 8]
                )
                nc.sync.dma_start(
                    out=out[:, :].rearrange("(b p) f -> p b f", p=128)[
                        :, qr * (RH // 128):(qr + 1) * (RH // 128), :
                    ],
                    in_=ob.rearrange("p (b f) -> p b f", f=FEAT),
                )
    nc.finalize()
    return nc


_NC_CACHE = {}


def _get_nc(R, C):
    key = (R, C)
    if key not in _NC_CACHE:
        _NC_CACHE[key] = _build_nc(R, C)
    return _NC_CACHE[key]


def kernel(x, y, y_atomflex, x_batch, y_batch):
    x = np.ascontiguousarray(np.asarray(x, dtype=np.float32))
    y = np.ascontiguousarray(np.asarray(y, dtype=np.float32))
    feats = np.ascontiguousarray(np.asarray(y_atomflex, dtype=np.float32))
    xb = np.asarray(x_batch).astype(np.int64)
    yb = np.asarray(y_batch).astype(np.int64)

    N = x.shape[0]
    R = N // N_CORES

    # per-core y spans (batch ids are sorted)
    spans = []
    for c in range(N_CORES):
        blo, bhi = xb[c * R], xb[(c + 1) * R - 1]
        s = int(np.searchsorted(yb, blo, "left"))
        e = int(np.searchsorted(yb, bhi, "right"))
        spans.append((s, e))
    C = max(1024, -(-max(e - s for s, e in spans) // 1024) * 1024)
    G = C // 128

    in_maps = []
    for c in range(N_CORES):
        s, e = spans[c]
        n = e - s
        yc = np.zeros((C, 3), np.float32)
        yc[:n] = y[s:e]
        ybc = np.full(C, SENTINEL)
        ybc[:n] = yb[s:e]
        fec = np.zeros((C, FEAT), np.float32)
        fec[:n] = feats[s:e]
        fe_bf = fec.astype(bf16)
        X, Y = _build_sides(x[c * R:(c + 1) * R], xb[c * R:(c + 1) * R], yc, ybc)
        # per-quarter colsum over the Sign (+/-1) chunks: out = (gT + cs)/32
        csq = np.zeros((FEAT, NH), np.float64)
        for qr in range(NH):
            mask = np.zeros(C, np.float64)
            for jc in range(G):
                if _act_chunk(jc, qr):
                    mask[jc * 128:(jc + 1) * 128] = 1.0
            csq[:, qr] = (fe_bf.astype(np.float64) * mask[:, None]).sum(0)
        in_maps.append(
            {
                "xk": np.ascontiguousarray(X),
                "yk": np.ascontiguousarray(Y),
                "fe": np.ascontiguousarray(fe_bf),
                "cs": np.ascontiguousarray(csq.astype(np.float32)),
            }
        )

    nc = _get_nc(R, C)
    trace = bool(int(os.environ.get("KNN_TRACE", "0")))
    res = run_bass_kernel_spmd(
        nc, in_maps, core_ids=list(range(N_CORES)), trace=trace
    )
    if trace and res.exec_time_ns is not None:
        print(f"HW exec time: {res.exec_time_ns} ns")
        if res.instructions_and_trace is not None:
            print(f"trace: {res.instructions_and_trace[1]}")

    out = np.concatenate([r["out"] for r in res.results], axis=0)
    return out.astype(np.float32)


if __name__ == "__main__":
    # smoke test against the local reference
    import reference

    inputs = {k: np.asarray(v) for k, v in reference.setup_inputs().items()}
    expected = np.asarray(reference.reference(**inputs))
    actual = kernel(**inputs)
    err = np.linalg.norm(actual - expected) / np.linalg.norm(expected)
    print(f"Relative error: {err:.6f}")
